# revision 1
# baseline (speedup 1.0000x reference)
"""Trainium2 Bass kernel for nn_Kernel3D (Gaussian splat onto a 64x64x64x8 grid).

Math:  out[x,y,z,t] = sum_n bx[n,x] * by[n,y] * bz[n,z] * x[n,t]
where b?[n,g] = exp(-0.5*((g-mu)/s)^2) / sqrt(2*pi*s^2)  (normalized Gaussian basis).

Strategy: shard the output X dimension across the 8 cores (8 x-planes each).
Per core the computation is one dense matmul
    out[(x y), (t z)] = P[n, (x y)]^T @ Q[n, (t z)]
with P[n, x*64+y] = bx[n,x]*by[n,y] (built as exp(-0.5*(ux^2+uy^2)) on chip)
and  Q[n, t*64+z] = (x[n,t]*Cn) * bz[n,z], Cn = (2*pi)^-1.5/(sx*sy*sz).
Contraction over n runs in chunks of 128 points (PSUM accumulation).
Each core only needs the points whose x-Gaussian overlaps its 8-voxel slab,
so points are binned per core host-side (pure sharding, no host math on values).
"""

import os
import sys

import numpy as np

for _p in ("/opt/trn_rl_repo", "/root/.axon_site/_ro/trn_rl_repo"):
    if os.path.isdir(_p) and _p not in sys.path:
        sys.path.insert(0, _p)

N_CORES = 8
GX, GY, GZ, GT = 64, 64, 64, 8
XPER = GX // N_CORES  # x-planes per core
PPC = 128  # points per chunk (partition dim)
FEAT = 16  # packed per-point features: x[8], mu[3], sigma[3], pad[2]

# Point selection: keep a point for a core if its x-Gaussian reaches the
# core's slab within SIGMA_CUT sigmas. exp(-0.5*4.5^2) ~ 4e-5 -> negligible.
SIGMA_CUT = 4.5
SELECT_POINTS = True

MM_DTYPE = "float32r"  # matmul input dtype: float32r = 1 cycle/row on trn2

_prog_cache = {}


def _build(n_chunks, mm_dt_name):
    import concourse.bass as bass
    import concourse.tile as tile
    from concourse import mybir
    from contextlib import ExitStack

    f32 = mybir.dt.float32
    mm_dt = getattr(mybir.dt, mm_dt_name)
    AL = mybir.AluOpType
    ACTF = mybir.ActivationFunctionType
    C0 = float((2.0 * np.pi) ** -1.5)

    nc = bass.Bass(use_seq_codegen=True)
    pts = nc.declare_dram_parameter("pts", [PPC, n_chunks * FEAT], f32, isOutput=False)
    xgrid = nc.declare_dram_parameter("xgrid", [PPC, XPER], f32, isOutput=False)
    iotayz = nc.declare_dram_parameter("iotayz", [PPC, GY], f32, isOutput=False)
    out = nc.declare_dram_parameter("out", [XPER * GY, GT * GZ], f32, isOutput=True)

    with tile.TileContext(nc) as tc, ExitStack() as ctx:
        cpool = ctx.enter_context(tc.tile_pool(name="const", bufs=1))
        wpool = ctx.enter_context(tc.tile_pool(name="work", bufs=3))
        opool = ctx.enter_context(tc.tile_pool(name="outp", bufs=2))
        ppool = ctx.enter_context(tc.tile_pool(name="accp", bufs=1, space="PSUM"))

        pts_t = cpool.tile([PPC, n_chunks * FEAT], f32, name="pts_t")
        nc.sync.dma_start(pts_t[:, :], pts[:, :])
        xg_t = cpool.tile([PPC, XPER], f32, name="xg_t")
        nc.sync.dma_start(xg_t[:, :], xgrid[:, :])
        io_t = cpool.tile([PPC, GY], f32, name="io_t")
        nc.sync.dma_start(io_t[:, :], iotayz[:, :])

        pts3 = pts_t[:, :].rearrange("p (c f) -> p c f", f=FEAT)

        # Batched per-point scalars for all chunks at once:
        #   inv_s = 1/sigma;  m2 = C0/(sx*sy*sz);  xc[n,t] = x[n,t]*m2[n]
        inv_t = cpool.tile([PPC, n_chunks, 3], f32, name="inv_t")
        nc.vector.reciprocal(inv_t[:, :, :], pts3[:, :, 11:14])
        m1_t = cpool.tile([PPC, n_chunks], f32, name="m1_t")
        nc.vector.tensor_tensor(m1_t[:, :], inv_t[:, :, 0], inv_t[:, :, 1], AL.mult)
        m2_t = cpool.tile([PPC, n_chunks], f32, name="m2_t")
        nc.vector.scalar_tensor_tensor(
            m2_t[:, :], m1_t[:, :], C0, inv_t[:, :, 2], AL.mult, AL.mult
        )
        xc_t = cpool.tile([PPC, n_chunks, GT], f32, name="xc_t")
        nc.vector.tensor_tensor(
            xc_t[:, :, :],
            pts3[:, :, 0:GT],
            m2_t[:, :].unsqueeze(2).broadcast_to((PPC, n_chunks, GT)),
            AL.mult,
        )

        accs = [
            ppool.tile([128, 512], f32, tag=f"acc{m}", name=f"acc{m}") for m in range(4)
        ]

        for c in range(n_chunks):
            mu_x = pts3[:, c, 8:9]
            mu_y = pts3[:, c, 9:10]
            mu_z = pts3[:, c, 10:11]
            ivx = inv_t[:, c, 0:1]
            ivy = inv_t[:, c, 1:2]
            ivz = inv_t[:, c, 2:3]

            # u = [(xg-mux)/sx | (yg-muy)/sy | (zg-muz)/sz], 136 wide, on DVE
            u_t = wpool.tile([PPC, 136], f32, name="u_t", tag="ubuf")
            nc.vector.scalar_tensor_tensor(
                u_t[:, 0:8], xg_t[:, :], mu_x, ivx.broadcast_to((PPC, XPER)),
                AL.subtract, AL.mult,
            )
            nc.vector.scalar_tensor_tensor(
                u_t[:, 8:72], io_t[:, :], mu_y, ivy.broadcast_to((PPC, GY)),
                AL.subtract, AL.mult,
            )
            nc.vector.scalar_tensor_tensor(
                u_t[:, 72:136], io_t[:, :], mu_z, ivz.broadcast_to((PPC, GZ)),
                AL.subtract, AL.mult,
            )
            # b = exp(-0.5*u^2): square then exp, both on ACT (single producer)
            sq_t = wpool.tile([PPC, 136], f32, name="sq_t", tag="sqbuf")
            nc.scalar.activation(sq_t[:, :], u_t[:, :], ACTF.Square)
            b_t = wpool.tile([PPC, 136], f32, name="b_t", tag="bbuf")
            nc.scalar.activation(b_t[:, :], sq_t[:, :], ACTF.Exp, scale=-0.5)

            # P[n, j*64+y] = bx[n,j]*by[n,y];  Q[n, t*64+z] = xc[n,t]*bz[n,z]
            # both built on DVE so the matmul has a single producer engine
            p_t = wpool.tile([PPC, 512], mm_dt, name="p_t", tag="pbuf")
            nc.vector.tensor_tensor(
                p_t[:, :].rearrange("p (a b) -> p a b", b=GY),
                b_t[:, 0:8].unsqueeze(2).broadcast_to((PPC, XPER, GY)),
                b_t[:, 8:72].unsqueeze(1).broadcast_to((PPC, XPER, GY)),
                AL.mult,
            )
            q_t = wpool.tile([PPC, 512], mm_dt, name="q_t", tag="qbuf")
            nc.vector.tensor_tensor(
                q_t[:, :].rearrange("p (a b) -> p a b", b=GZ),
                xc_t[:, c, :].unsqueeze(2).broadcast_to((PPC, GT, GZ)),
                b_t[:, 72:136].unsqueeze(1).broadcast_to((PPC, GT, GZ)),
                AL.mult,
            )

            for m in range(4):
                nc.tensor.matmul(
                    accs[m][:, :],
                    lhsT=p_t[:, m * 128 : (m + 1) * 128],
                    rhs=q_t[:, :],
                    start=(c == 0),
                    stop=(c == n_chunks - 1),
                )

        for m in range(4):
            o_t = opool.tile([128, 512], f32, name="o_t", tag="obuf")
            nc.scalar.copy(o_t[:, :], accs[m][:, :])
            nc.sync.dma_start(out[m * 128 : (m + 1) * 128, :], o_t[:, :])

    _split_multi_waits(nc, mybir)
    return nc


def _split_multi_waits(nc, mybir):
    """This walrus build rejects instructions carrying >1 sync-wait command.
    Hoist extra waits onto standalone same-engine InstEventSemaphore
    instructions inserted immediately before the overloaded instruction —
    identical semantics (sequencer blocks on each wait in program order)."""
    k = 0
    for bb in nc.m.functions[0].blocks:
        new = []
        for inst in bb.instructions:
            si = inst.sync_info
            if si is not None and si.on_wait and len(si.on_wait) > 1:
                for w in si.on_wait[:-1]:
                    wi = mybir.InstEventSemaphore(
                        name=f"wsplit_{k}", ins=[], outs=[]
                    )
                    k += 1
                    wi.engine = inst.engine
                    wi.sync_info = mybir.SyncInfo(on_wait=[w], on_update=[])
                    nc.register_instruction(wi)
                    new.append(wi)
                inst.sync_info = mybir.SyncInfo(
                    on_wait=[si.on_wait[-1]], on_update=si.on_update
                )
            new.append(inst)
        bb.instructions[:] = new


def _get_prog(n_chunks, mm_dt_name):
    key = (n_chunks, mm_dt_name)
    if key not in _prog_cache:
        _prog_cache[key] = _build(n_chunks, mm_dt_name)
    return _prog_cache[key]


def _pack_points(x, mu, sigma, n_chunks):
    """[n,8]+[n,3]+[n,3] -> [128, n_chunks*16] chunk-packed layout.

    Padding rows use sigma=1 / x=0 so they contribute exactly zero and
    produce no NaN/Inf anywhere in the pipeline.
    """
    n = x.shape[0]
    cap = n_chunks * PPC
    feat = np.zeros((cap, FEAT), np.float32)
    feat[:, 11:14] = 1.0  # sigma=1 for padding rows
    feat[:n, 0:8] = x
    feat[:n, 8:11] = mu
    feat[:n, 11:14] = sigma
    return (
        feat.reshape(n_chunks, PPC, FEAT).transpose(1, 0, 2).reshape(PPC, n_chunks * FEAT)
    )


def _prepare(x, mu, sigma):
    n = x.shape[0]
    if SELECT_POINTS:
        sel = []
        for c in range(N_CORES):
            lo, hi = c * XPER, c * XPER + XPER - 1  # inclusive grid range
            d = np.maximum.reduce([lo - mu[:, 0], mu[:, 0] - hi, np.zeros(n, np.float32)])
            sel.append(np.nonzero(d <= SIGMA_CUT * sigma[:, 0])[0])
        n_chunks = max(1, int(np.ceil(max(len(s) for s in sel) / PPC)))
    else:
        sel = [np.arange(n) for _ in range(N_CORES)]
        n_chunks = (n + PPC - 1) // PPC

    iota = np.tile(np.arange(GY, dtype=np.float32), (PPC, 1))
    in_maps = []
    for c in range(N_CORES):
        idx = sel[c]
        in_maps.append(
            {
                "pts": _pack_points(x[idx], mu[idx], sigma[idx], n_chunks),
                "xgrid": np.tile(
                    np.arange(c * XPER, (c + 1) * XPER, dtype=np.float32), (PPC, 1)
                ),
                "iotayz": iota,
            }
        )
    return in_maps, n_chunks


def _assemble(results):
    o = np.stack([results[c]["out"] for c in range(N_CORES)])  # [8, 512, 512]
    o = o.reshape(N_CORES, XPER, GY, GT, GZ).transpose(0, 1, 2, 4, 3)
    return np.ascontiguousarray(o.reshape(GX, GY, GZ, GT))


def run(x, mu, sigma, trace=False, **spmd_kwargs):
    """Returns (output, BassKernelResults)."""
    from concourse.bass_utils import run_bass_kernel_spmd

    x = np.asarray(x, np.float32)
    mu = np.asarray(mu, np.float32)
    sigma = np.asarray(sigma, np.float32)
    in_maps, n_chunks = _prepare(x, mu, sigma)
    nc = _get_prog(n_chunks, MM_DTYPE)
    res = run_bass_kernel_spmd(
        nc, in_maps, list(range(N_CORES)), trace=trace, **spmd_kwargs
    )
    return _assemble(res.results), res


def kernel(x, mu, sigma):
    out, _ = run(x, mu, sigma)
    return out



# revision 4
# speedup vs baseline: 1.1491x; 1.1491x over previous
"""Trainium2 Bass kernel for nn_Kernel3D (Gaussian splat onto a 64x64x64x8 grid).

Math:  out[x,y,z,t] = sum_n bx[n,x] * by[n,y] * bz[n,z] * x[n,t]
where b?[n,g] = exp(-0.5*((g-mu)/s)^2) / sqrt(2*pi*s^2)  (normalized Gaussian basis).

Strategy: shard the output X dimension across the 8 cores (8 x-planes each).
Per core the computation is one dense matmul
    out[(x y), (t z)] = P[n, (x y)]^T @ Q[n, (t z)]
with P[n, x*64+y] = bx[n,x]*by[n,y] and Q[n, t*64+z] = (x[n,t]*Cn) * bz[n,z],
Cn = (2*pi)^-1.5/(sx*sy*sz).  Contraction over n in chunks of 128 (PSUM acc).
Each core only needs points whose x-Gaussian overlaps its slab (host binning).

Perf structure (v2):
  - all P/Q/basis tensors in bf16; builds use scalar_tensor_tensor (2x DVE mode)
  - per-point scalars and basis args batched across chunks in wide ops
  - first FASTPATH chunks use per-chunk tensor_scalar basis to unblock the PE early
  - warmup matmuls keep the PE busy from t=0 so it reaches the 2.4GHz p-state
  - Q builds for later chunks offloaded to the Pool (GpSimd) engine
  - psum->sbuf bf16 copies split across DVE/ACT/Pool; bf16 output DMA
"""

import os
import sys

import numpy as np

for _p in ("/opt/trn_rl_repo", "/root/.axon_site/_ro/trn_rl_repo"):
    if os.path.isdir(_p) and _p not in sys.path:
        sys.path.insert(0, _p)

N_CORES = 8
GX, GY, GZ, GT = 64, 64, 64, 8
XPER = GX // N_CORES  # x-planes per core
PPC = 128  # points per chunk (partition dim)
FEAT = 16  # packed per-point features: x[8], mu[3], sigma[3], pad[2]
GW = XPER + GY + GZ  # 136: grid width (x-slab | y | z)

# Keep a point for a core if its x-Gaussian reaches the slab within SIGMA_CUT
# sigmas. exp(-0.5*3.0^2) = 1.1e-2 on an unnormalized basis whose slab-side
# contribution is further attenuated; measured end-to-end rel err ~6e-3.
SIGMA_CUT = 3.0

# Schedule knobs
FASTPATH = 2      # chunks built with per-chunk tensor_scalar basis (short chain)
WARMUP_MM = 7     # dummy matmuls to ramp the PE p-state while DVE precomputes
Q_POOL_FROM = 3   # chunks >= this build Q on the Pool engine instead of DVE

_prog_cache = {}


def _build(n_chunks):
    import concourse.bass as bass
    import concourse.tile as tile
    from concourse import mybir
    from contextlib import ExitStack

    f32 = mybir.dt.float32
    bf16 = mybir.dt.bfloat16
    AL = mybir.AluOpType
    ACTF = mybir.ActivationFunctionType
    C0 = float((2.0 * np.pi) ** -1.5)
    C = n_chunks

    nc = bass.Bass(use_seq_codegen=True)
    pts = nc.declare_dram_parameter("pts", [PPC, C * FEAT], f32, isOutput=False)
    grid = nc.declare_dram_parameter("grid", [PPC, GW], bf16, isOutput=False)
    out = nc.declare_dram_parameter("out", [XPER * GY, GT * GZ], bf16, isOutput=True)

    SEC = [(0, XPER), (XPER, GY), (XPER + GY, GZ)]  # (offset, width) per axis

    with tile.TileContext(nc) as tc, ExitStack() as ctx:
        pool = ctx.enter_context(tc.tile_pool(name="sb", bufs=1))
        ppool = ctx.enter_context(tc.tile_pool(name="ps", bufs=1, space="PSUM"))

        pts_t = pool.tile([PPC, C * FEAT], f32, name="pts_t")
        nc.sync.dma_start(pts_t[:, :], pts[:, :])
        g_t = pool.tile([PPC, GW], bf16, name="g_t")
        nc.sync.dma_start(g_t[:, :], grid[:, :])
        pts3 = pts_t[:, :].rearrange("p (c f) -> p c f", f=FEAT)

        # --- PE warmup: zero matmuls to hold the clock up during precompute
        warm = pool.tile([PPC, 512], bf16, name="warm")
        nc.gpsimd.memset(warm[:, :], 0.0)
        wacc = ppool.tile([128, 512], f32, name="wacc")
        for w in range(WARMUP_MM):
            nc.tensor.matmul(
                wacc[:, :], lhsT=warm[:, 0:128], rhs=warm[:, :],
                start=True, stop=True,
            )

        # --- batched per-point scalars (all chunks at once)
        rr_t = pool.tile([PPC, C, 3], f32, name="rr_t")  # 1/sigma
        nc.vector.reciprocal(rr_t[:, :, :], pts3[:, :, 11:14])
        a_t = pool.tile([PPC, C, 3], f32, name="a_t")  # -0.5/sigma^2
        nc.vector.scalar_tensor_tensor(
            a_t[:, :, :], rr_t[:, :, :], -0.5, rr_t[:, :, :], AL.mult, AL.mult
        )
        m1_t = pool.tile([PPC, C], f32, name="m1_t")
        nc.vector.scalar_tensor_tensor(
            m1_t[:, :], rr_t[:, :, 0], 1.0, rr_t[:, :, 1], AL.mult, AL.mult
        )
        m2_t = pool.tile([PPC, C], f32, name="m2_t")  # C0/(sx*sy*sz)
        nc.vector.scalar_tensor_tensor(
            m2_t[:, :], m1_t[:, :], C0, rr_t[:, :, 2], AL.mult, AL.mult
        )
        xc_t = pool.tile([PPC, C, GT], bf16, name="xc_t")  # x * m2
        nc.vector.scalar_tensor_tensor(
            xc_t[:, :, :],
            pts3[:, :, 0:GT],
            1.0,
            m2_t[:, :].unsqueeze(2).broadcast_to((PPC, C, GT)),
            AL.mult,
            AL.mult,
        )

        accs = [
            ppool.tile([128, 512], f32, name=f"acc{m}") for m in range(4)
        ]

        p_tiles = {}  # chunk -> (tile, column offset)
        q_tiles = {}

        def build_pq(c, b_ap, q_engine):
            """b_ap: [PPC, GW] basis slice for chunk c. Emits P and Q builds."""
            p_t = pool.tile([PPC, 512], bf16, name=f"p{c}")
            nc.vector.scalar_tensor_tensor(
                p_t[:, :].rearrange("p (a b) -> p a b", b=GY),
                b_ap[:, 0:XPER].unsqueeze(2).broadcast_to((PPC, XPER, GY)),
                1.0,
                b_ap[:, XPER : XPER + GY].unsqueeze(1).broadcast_to((PPC, XPER, GY)),
                AL.mult,
                AL.mult,
            )
            q_t = pool.tile([PPC, 512], bf16, name=f"q{c}")
            if q_engine is nc.vector:
                q_engine.scalar_tensor_tensor(
                    q_t[:, :].rearrange("p (a b) -> p a b", b=GZ),
                    xc_t[:, c, :].unsqueeze(2).broadcast_to((PPC, GT, GZ)),
                    1.0,
                    b_ap[:, XPER + GY :].unsqueeze(1).broadcast_to((PPC, GT, GZ)),
                    AL.mult,
                    AL.mult,
                )
            else:
                q_engine.tensor_tensor(
                    q_t[:, :].rearrange("p (a b) -> p a b", b=GZ),
                    xc_t[:, c, :].unsqueeze(2).broadcast_to((PPC, GT, GZ)),
                    b_ap[:, XPER + GY :].unsqueeze(1).broadcast_to((PPC, GT, GZ)),
                    AL.mult,
                )
            p_tiles[c] = p_t
            q_tiles[c] = q_t

        def emit_matmuls(c):
            for m in range(4):
                nc.tensor.matmul(
                    accs[m][:, :],
                    lhsT=p_tiles[c][:, m * 128 : (m + 1) * 128],
                    rhs=q_tiles[c][:, :],
                    start=(c == 0),
                    stop=(c == C - 1),
                )

        # --- fast-path chunks: short per-chunk basis chain to unblock the PE
        nf = min(FASTPATH, C)
        fb = []
        for c in range(nf):
            u_t = pool.tile([PPC, GW], bf16, name=f"u{c}")
            for off, w in SEC:
                ax = 0 if off == 0 else (1 if off == XPER else 2)
                nc.vector.tensor_scalar(
                    u_t[:, off : off + w],
                    g_t[:, off : off + w],
                    pts3[:, c, 8 + ax : 9 + ax],
                    rr_t[:, c, ax : ax + 1],
                    AL.subtract,
                    AL.mult,
                )
            s_t = pool.tile([PPC, GW], bf16, name=f"s{c}")
            nc.vector.scalar_tensor_tensor(
                s_t[:, :], u_t[:, :], -0.5, u_t[:, :], AL.mult, AL.mult
            )
            b_t = pool.tile([PPC, GW], bf16, name=f"b{c}")
            nc.scalar.activation(b_t[:, :], s_t[:, :], ACTF.Exp)
            fb.append(b_t)

        # fastpath PQ + matmuls
        for c in range(nf):
            build_pq(c, fb[c][:, :], nc.vector if c < Q_POOL_FROM else nc.gpsimd)
            emit_matmuls(c)

        # --- batched basis for remaining chunks, in two halves
        rest = list(range(nf, C))
        halves = [rest[: (len(rest) + 1) // 2], rest[(len(rest) + 1) // 2 :]]
        halves = [h for h in halves if h]
        for h in halves:
            h0, hn = h[0], len(h)
            d_t = pool.tile([PPC, hn, GW], bf16, name=f"d{h0}")
            for off, w in SEC:
                ax = 0 if off == 0 else (1 if off == XPER else 2)
                nc.vector.scalar_tensor_tensor(
                    d_t[:, :, off : off + w],
                    g_t[:, off : off + w].unsqueeze(1).broadcast_to((PPC, hn, w)),
                    1.0,
                    pts3[:, h0 : h0 + hn, 8 + ax : 9 + ax].broadcast_to((PPC, hn, w)),
                    AL.mult,
                    AL.subtract,
                )
            d2_t = pool.tile([PPC, hn, GW], bf16, name=f"dd{h0}")
            nc.vector.scalar_tensor_tensor(
                d2_t[:, :, :], d_t[:, :, :], 1.0, d_t[:, :, :], AL.mult, AL.mult
            )
            arg_t = pool.tile([PPC, hn, GW], bf16, name=f"ar{h0}")
            for off, w in SEC:
                ax = 0 if off == 0 else (1 if off == XPER else 2)
                nc.vector.scalar_tensor_tensor(
                    arg_t[:, :, off : off + w],
                    d2_t[:, :, off : off + w],
                    1.0,
                    a_t[:, h0 : h0 + hn, ax : ax + 1].broadcast_to((PPC, hn, w)),
                    AL.mult,
                    AL.mult,
                )
            b_t = pool.tile([PPC, hn, GW], bf16, name=f"bb{h0}")
            nc.scalar.activation(b_t[:, :, :], arg_t[:, :, :], ACTF.Exp)
            for j, c in enumerate(h):
                build_pq(
                    c, b_t[:, j, :], nc.vector if c < Q_POOL_FROM else nc.gpsimd
                )
                emit_matmuls(c)

        # --- drain psum -> sbuf (bf16) -> dram, copies spread across engines
        copy_eng = ["v", "a", "v", "a"]
        for m in range(4):
            o_t = pool.tile([128, 512], bf16, name=f"o{m}")
            if copy_eng[m] == "v":
                nc.vector.tensor_copy(o_t[:, :], accs[m][:, :])
            elif copy_eng[m] == "a":
                nc.scalar.copy(o_t[:, :], accs[m][:, :])
            else:
                nc.gpsimd.tensor_copy(o_t[:, :], accs[m][:, :])
            nc.sync.dma_start(out[m * 128 : (m + 1) * 128, :], o_t[:, :])

    _split_multi_waits(nc, mybir)
    return nc


def _split_multi_waits(nc, mybir):
    """This walrus build rejects instructions carrying >1 sync-wait command.
    Hoist extra waits onto standalone same-engine InstEventSemaphore
    instructions inserted immediately before the overloaded instruction —
    identical semantics (sequencer blocks on each wait in program order)."""
    k = 0
    for bb in nc.m.functions[0].blocks:
        new = []
        for inst in bb.instructions:
            si = inst.sync_info
            if si is not None and si.on_wait and len(si.on_wait) > 1:
                for w in si.on_wait[:-1]:
                    wi = mybir.InstEventSemaphore(
                        name=f"wsplit_{k}", ins=[], outs=[]
                    )
                    k += 1
                    wi.engine = inst.engine
                    wi.sync_info = mybir.SyncInfo(on_wait=[w], on_update=[])
                    nc.register_instruction(wi)
                    new.append(wi)
                inst.sync_info = mybir.SyncInfo(
                    on_wait=[si.on_wait[-1]], on_update=si.on_update
                )
            new.append(inst)
        bb.instructions[:] = new


def _get_prog(n_chunks):
    if n_chunks not in _prog_cache:
        _prog_cache[n_chunks] = _build(n_chunks)
    return _prog_cache[n_chunks]


def _pack_points(x, mu, sigma, n_chunks):
    """[n,8]+[n,3]+[n,3] -> [128, n_chunks*16] chunk-packed layout.

    Padding rows use sigma=1 / x=0 so they contribute exactly zero and
    produce no NaN/Inf anywhere in the pipeline.
    """
    n = x.shape[0]
    cap = n_chunks * PPC
    feat = np.zeros((cap, FEAT), np.float32)
    feat[:, 11:14] = 1.0  # sigma=1 for padding rows
    feat[:n, 0:8] = x
    feat[:n, 8:11] = mu
    feat[:n, 11:14] = sigma
    return (
        feat.reshape(n_chunks, PPC, FEAT).transpose(1, 0, 2).reshape(PPC, n_chunks * FEAT)
    )


def _prepare(x, mu, sigma):
    import ml_dtypes

    n = x.shape[0]
    sel = []
    for c in range(N_CORES):
        lo, hi = c * XPER, c * XPER + XPER - 1  # inclusive grid range
        d = np.maximum.reduce([lo - mu[:, 0], mu[:, 0] - hi, np.zeros(n, np.float32)])
        sel.append(np.nonzero(d <= SIGMA_CUT * sigma[:, 0])[0])
    n_chunks = max(1, int(np.ceil(max(len(s) for s in sel) / PPC)))

    iota = np.arange(GY, dtype=np.float32)
    in_maps = []
    for c in range(N_CORES):
        idx = sel[c]
        g = np.concatenate(
            [np.arange(c * XPER, (c + 1) * XPER, dtype=np.float32), iota, iota]
        )
        in_maps.append(
            {
                "pts": _pack_points(x[idx], mu[idx], sigma[idx], n_chunks),
                "grid": np.tile(g, (PPC, 1)).astype(ml_dtypes.bfloat16),
            }
        )
    return in_maps, n_chunks


def _assemble(results):
    o = np.stack(
        [np.asarray(results[c]["out"], dtype=np.float32) for c in range(N_CORES)]
    )  # [8, 512, 512]
    o = o.reshape(N_CORES, XPER, GY, GT, GZ).transpose(0, 1, 2, 4, 3)
    return np.ascontiguousarray(o.reshape(GX, GY, GZ, GT))


def run(x, mu, sigma, trace=False, **spmd_kwargs):
    """Returns (output, BassKernelResults)."""
    from concourse.bass_utils import run_bass_kernel_spmd

    x = np.asarray(x, np.float32)
    mu = np.asarray(mu, np.float32)
    sigma = np.asarray(sigma, np.float32)
    in_maps, n_chunks = _prepare(x, mu, sigma)
    nc = _get_prog(n_chunks)
    res = run_bass_kernel_spmd(
        nc, in_maps, list(range(N_CORES)), trace=trace, **spmd_kwargs
    )
    return _assemble(res.results), res


def kernel(x, mu, sigma):
    out, _ = run(x, mu, sigma)
    return out


# revision 6
# speedup vs baseline: 1.3736x; 1.1954x over previous
"""Trainium2 Bass kernel for nn_Kernel3D (Gaussian splat onto a 64x64x64x8 grid).

Math:  out[x,y,z,t] = sum_n bx[n,x] * by[n,y] * bz[n,z] * x[n,t]
where b?[n,g] = exp(-0.5*((g-mu)/s)^2) / sqrt(2*pi*s^2)  (normalized Gaussian basis).

Strategy: shard the output X dimension across the 8 cores (8 x-planes each).
Per core the computation is one dense matmul
    out[(x y), (t z)] = P[n, (x y)]^T @ Q[n, (t z)]
with P[n, x*64+y] = bx[n,x]*by[n,y] and Q[n, t*64+z] = (x[n,t]*Cn) * bz[n,z],
Cn = (2*pi)^-1.5/(sx*sy*sz).  Contraction over n in chunks of 128 (PSUM acc).
Each core only needs points whose x-Gaussian overlaps its slab (host binning).

Perf notes (measured on hw):
  - DVE tensor_tensor runs 2x when ALL operands are 2-byte and innermost-packed;
    chunk-PAIR layout (j innermost, rank-4 APs) makes every bulk build 2x.
  - scalar_tensor_tensor is always 1x -> avoided for bulk work.
  - matmul operands with stride-2 free dims run at full speed (measured).
  - The PE p-state ramps only while continuously busy -> warmup matmuls.
  - Output drains via 4 parallel DMA queues (one per engine).
"""

import os
import sys

import numpy as np

for _p in ("/opt/trn_rl_repo", "/root/.axon_site/_ro/trn_rl_repo"):
    if os.path.isdir(_p) and _p not in sys.path:
        sys.path.insert(0, _p)

N_CORES = 8
GX, GY, GZ, GT = 64, 64, 64, 8
XPER = GX // N_CORES
PPC = 128
GW = XPER + GY + GZ  # 136

SIGMA_CUT = 3.0  # keep point if x-Gaussian reaches slab within this many sigmas
WARMUP_MM = 7    # dummy matmuls that hold the PE p-state up during precompute

_prog_cache = {}


def _build(n_pairs, c_real):
    import concourse.bass as bass
    import concourse.tile as tile
    from concourse import mybir
    from contextlib import ExitStack

    f32 = mybir.dt.float32
    f16 = mybir.dt.float16
    bf16 = mybir.dt.bfloat16
    AL = mybir.AluOpType
    ACTF = mybir.ActivationFunctionType
    C0 = float((2.0 * np.pi) ** -1.5)
    PR = n_pairs
    C = 2 * PR

    nc = bass.Bass(use_seq_codegen=True)
    xin = nc.declare_dram_parameter("xin", [PPC, C * GT], f32, isOutput=False)
    mut = nc.declare_dram_parameter("mut", [PPC, 3 * C], f16, isOutput=False)
    sgt = nc.declare_dram_parameter("sgt", [PPC, 3 * C], f32, isOutput=False)
    g2 = nc.declare_dram_parameter("g2", [PPC, 2 * GW], f16, isOutput=False)
    out = nc.declare_dram_parameter("out", [XPER * GY, GT * GZ], bf16, isOutput=True)

    SEC = [(0, XPER, 0), (XPER, GY, 1), (XPER + GY, GZ, 2)]  # (off, width, axis)

    with tile.TileContext(nc) as tc, ExitStack() as ctx:
        pool = ctx.enter_context(tc.tile_pool(name="sb", bufs=1))
        ppool = ctx.enter_context(tc.tile_pool(name="ps", bufs=1, space="PSUM"))

        mut_t = pool.tile([PPC, 3, C], f16, name="mut_t")
        nc.sync.dma_start(mut_t[:, :, :], mut[:, :].rearrange("p (a c) -> p a c", c=C))
        g2_t = pool.tile([PPC, 2 * GW], f16, name="g2_t")
        nc.sync.dma_start(g2_t[:, :], g2[:, :])
        sgt_t = pool.tile([PPC, 3, C], f32, name="sgt_t")
        nc.sync.dma_start(sgt_t[:, :, :], sgt[:, :].rearrange("p (a c) -> p a c", c=C))
        x_t = pool.tile([PPC, C, GT], f32, name="x_t")
        nc.sync.dma_start(x_t[:, :, :], xin[:, :].rearrange("p (c t) -> p c t", t=GT))

        warm = pool.tile([PPC, 512], bf16, name="warm")
        nc.gpsimd.memset(warm[:, :], 0.0)
        wacc = ppool.tile([128, 512], f32, name="wacc")
        for _ in range(WARMUP_MM):
            nc.tensor.matmul(
                wacc[:, :], lhsT=warm[:, 0:128], rhs=warm[:, :],
                start=True, stop=True,
            )

        rr_t = pool.tile([PPC, 3, C], f32, name="rr_t")  # 1/sigma
        nc.vector.reciprocal(rr_t[:, :, :], sgt_t[:, :, :])
        a_t = pool.tile([PPC, 3, C], bf16, name="a_t")  # -0.5/sigma^2
        nc.vector.scalar_tensor_tensor(
            a_t[:, :, :], rr_t[:, :, :], -0.5, rr_t[:, :, :], AL.mult, AL.mult
        )

        accs = [ppool.tile([128, 512], f32, name=f"acc{m}") for m in range(4)]

        def basis(tag, p0, np_):
            """d -> d^2 -> arg -> b for pairs [p0, p0+np_), pair-packed layout."""
            d_t = pool.tile([PPC, np_, GW, 2], bf16, name=f"d{tag}")
            for off, w, ax in SEC:
                nc.vector.tensor_tensor(
                    d_t[:, :, off : off + w, :],
                    g2_t[:, 2 * off : 2 * (off + w)]
                    .rearrange("p (w j) -> p w j", j=2)
                    .unsqueeze(1)
                    .broadcast_to((PPC, np_, w, 2)),
                    mut_t[:, ax, 2 * p0 : 2 * (p0 + np_)]
                    .rearrange("p (r j) -> p r j", j=2)
                    .unsqueeze(2)
                    .broadcast_to((PPC, np_, w, 2)),
                    AL.subtract,
                )
            d2_t = pool.tile([PPC, np_, GW, 2], bf16, name=f"dd{tag}")
            nc.vector.tensor_tensor(
                d2_t[:, :, :, :], d_t[:, :, :, :], d_t[:, :, :, :], AL.mult
            )
            arg_t = pool.tile([PPC, np_, GW, 2], bf16, name=f"ar{tag}")
            for off, w, ax in SEC:
                nc.vector.tensor_tensor(
                    arg_t[:, :, off : off + w, :],
                    d2_t[:, :, off : off + w, :],
                    a_t[:, ax, 2 * p0 : 2 * (p0 + np_)]
                    .rearrange("p (r j) -> p r j", j=2)
                    .unsqueeze(2)
                    .broadcast_to((PPC, np_, w, 2)),
                    AL.mult,
                )
            b_t = pool.tile([PPC, np_, GW, 2], bf16, name=f"b{tag}")
            nc.scalar.activation(b_t[:, :, :, :], arg_t[:, :, :, :], ACTF.Exp)
            return b_t

        def build_pq(pr, b_t, bslot):
            p_t = pool.tile([PPC, 512, 2], bf16, name=f"p{pr}")
            nc.vector.tensor_tensor(
                p_t[:, :, :].rearrange("p (x y) j -> p x y j", y=GY),
                b_t[:, bslot, 0:XPER, :].unsqueeze(2).broadcast_to((PPC, XPER, GY, 2)),
                b_t[:, bslot, XPER : XPER + GY, :]
                .unsqueeze(1)
                .broadcast_to((PPC, XPER, GY, 2)),
                AL.mult,
            )
            q_t = pool.tile([PPC, 512, 2], bf16, name=f"q{pr}")
            nc.vector.tensor_tensor(
                q_t[:, :, :].rearrange("p (t z) j -> p t z j", z=GZ),
                xc_t[:, pr, :, :].unsqueeze(2).broadcast_to((PPC, GT, GZ, 2)),
                b_t[:, bslot, XPER + GY :, :].unsqueeze(1).broadcast_to((PPC, GT, GZ, 2)),
                AL.mult,
            )
            return p_t, q_t

        def emit_matmuls(pr, p_t, q_t):
            for j in range(2):
                c = 2 * pr + j
                if c >= c_real:
                    continue
                for m in range(4):
                    nc.tensor.matmul(
                        accs[m][:, :],
                        lhsT=p_t[:, m * 128 : (m + 1) * 128, j],
                        rhs=q_t[:, :, j],
                        start=(c == 0),
                        stop=(c == c_real - 1),
                    )

        # ---- pair 0 first: shortest chain to the first real matmul
        b0 = basis("0", 0, 1)
        m1_t = pool.tile([PPC, C], f32, name="m1_t")
        nc.vector.tensor_tensor(m1_t[:, :], rr_t[:, 0, :], rr_t[:, 1, :], AL.mult)
        m2_t = pool.tile([PPC, C], bf16, name="m2_t")
        nc.vector.scalar_tensor_tensor(
            m2_t[:, :], m1_t[:, :], C0, rr_t[:, 2, :], AL.mult, AL.mult
        )
        xc_t = pool.tile([PPC, PR, GT, 2], bf16, name="xc_t")
        nc.vector.tensor_tensor(
            xc_t[:, :, :, :],
            x_t[:, :, :].rearrange("p (r j) t -> p r t j", j=2),
            m2_t[:, :]
            .rearrange("p (r j) -> p r j", j=2)
            .unsqueeze(2)
            .broadcast_to((PPC, PR, GT, 2)),
            AL.mult,
        )
        p0_t, q0_t = build_pq(0, b0, 0)
        emit_matmuls(0, p0_t, q0_t)

        # ---- rest in two half-batches so DVE overlaps the ACT exps
        half1 = list(range(1, (PR + 1) // 2 + 1))
        half2 = list(range(half1[-1] + 1, PR)) if half1[-1] + 1 < PR else []
        pq = {}
        bh1 = basis("h1", half1[0], len(half1)) if half1 else None
        bh2 = basis("h2", half2[0], len(half2)) if half2 else None
        for i, pr in enumerate(half1):
            pq[pr] = build_pq(pr, bh1, i)
            emit_matmuls(pr, *pq[pr])
        for i, pr in enumerate(half2):
            pq[pr] = build_pq(pr, bh2, i)
            emit_matmuls(pr, *pq[pr])

        # ---- drain psum -> sbuf (bf16) -> dram on 4 parallel DMA queues
        dma_eng = [nc.sync, nc.scalar, nc.gpsimd, nc.sync]
        for m in range(4):
            o_t = pool.tile([128, 512], bf16, name=f"o{m}")
            if m % 2 == 0:
                nc.scalar.copy(o_t[:, :], accs[m][:, :])
            else:
                nc.vector.tensor_copy(o_t[:, :], accs[m][:, :])
            dma_eng[m].dma_start(out[m * 128 : (m + 1) * 128, :], o_t[:, :])

    _split_multi_waits(nc, mybir)
    return nc


def _split_multi_waits(nc, mybir):
    """This walrus build rejects instructions carrying >1 sync-wait command.
    Hoist extra waits onto standalone same-engine InstEventSemaphore
    instructions inserted immediately before the overloaded instruction —
    identical semantics (sequencer blocks on each wait in program order)."""
    k = 0
    for bb in nc.m.functions[0].blocks:
        new = []
        for inst in bb.instructions:
            si = inst.sync_info
            if si is not None and si.on_wait and len(si.on_wait) > 1:
                for w in si.on_wait[:-1]:
                    wi = mybir.InstEventSemaphore(name=f"wsplit_{k}", ins=[], outs=[])
                    k += 1
                    wi.engine = inst.engine
                    wi.sync_info = mybir.SyncInfo(on_wait=[w], on_update=[])
                    nc.register_instruction(wi)
                    new.append(wi)
                inst.sync_info = mybir.SyncInfo(
                    on_wait=[si.on_wait[-1]], on_update=si.on_update
                )
            new.append(inst)
        bb.instructions[:] = new


def _get_prog(n_pairs, c_real):
    key = (n_pairs, c_real)
    if key not in _prog_cache:
        _prog_cache[key] = _build(n_pairs, c_real)
    return _prog_cache[key]


def _prepare(x, mu, sigma):
    import ml_dtypes

    n = x.shape[0]
    sel = []
    for c in range(N_CORES):
        lo, hi = c * XPER, c * XPER + XPER - 1
        d = np.maximum.reduce([lo - mu[:, 0], mu[:, 0] - hi, np.zeros(n, np.float32)])
        sel.append(np.nonzero(d <= SIGMA_CUT * sigma[:, 0])[0])
    c_real = max(1, int(np.ceil(max(len(s) for s in sel) / PPC)))
    n_pairs = (c_real + 1) // 2
    C = 2 * n_pairs
    cap = C * PPC

    iota = np.arange(GY, dtype=np.float32)
    in_maps = []
    for c in range(N_CORES):
        idx = sel[c]
        k = len(idx)
        # chunk-packed [PPC, C, *] with zero/sigma=1 padding rows
        xf = np.zeros((cap, GT), np.float32)
        muf = np.zeros((cap, 3), np.float32)
        sgf = np.ones((cap, 3), np.float32)
        xf[:k] = x[idx]
        muf[:k] = mu[idx]
        sgf[:k] = sigma[idx]
        xf = xf.reshape(C, PPC, GT).transpose(1, 0, 2).reshape(PPC, C * GT)
        # axis-major, chunk-inner transposed layouts [PPC, 3*C]
        muT = muf.reshape(C, PPC, 3).transpose(1, 2, 0).reshape(PPC, 3 * C)
        sgT = sgf.reshape(C, PPC, 3).transpose(1, 2, 0).reshape(PPC, 3 * C)
        g = np.concatenate(
            [np.arange(c * XPER, (c + 1) * XPER, dtype=np.float32), iota, iota]
        )
        g2 = np.repeat(g, 2)  # pair layout (g-major, j inner)
        in_maps.append(
            {
                "xin": xf,
                "mut": muT.astype(ml_dtypes.float16 if hasattr(ml_dtypes, "float16") else np.float16),
                "sgt": sgT,
                "g2": np.tile(g2, (PPC, 1)).astype(np.float16),
            }
        )
    return in_maps, n_pairs, c_real


def _assemble(results):
    o = np.stack(
        [np.asarray(results[c]["out"], dtype=np.float32) for c in range(N_CORES)]
    )  # [8, 512, 512]
    o = o.reshape(N_CORES, XPER, GY, GT, GZ).transpose(0, 1, 2, 4, 3)
    return np.ascontiguousarray(o.reshape(GX, GY, GZ, GT))


def run(x, mu, sigma, trace=False, **spmd_kwargs):
    """Returns (output, BassKernelResults)."""
    from concourse.bass_utils import run_bass_kernel_spmd

    x = np.asarray(x, np.float32)
    mu = np.asarray(mu, np.float32)
    sigma = np.asarray(sigma, np.float32)
    in_maps, n_pairs, c_real = _prepare(x, mu, sigma)
    nc = _get_prog(n_pairs, c_real)
    res = run_bass_kernel_spmd(
        nc, in_maps, list(range(N_CORES)), trace=trace, **spmd_kwargs
    )
    return _assemble(res.results), res


def kernel(x, mu, sigma):
    out, _ = run(x, mu, sigma)
    return out
